# revision 13
# baseline (speedup 1.0000x reference)
"""Quantized Linear (8-bit act / 4-bit weight fake-quant) on 8 Trainium2 cores.

Math (per reference):
  xq = rne(x / s_x) * s_x          s_x = max(absmax(x)/127, 1e-8)
  wq = rne(w / s_w) * s_w          s_w = max(absmax(w)/7,   1e-8)
  bq = rne(b / s_b) * s_b          s_b = max(absmax(b)/127, 1e-8)
  out_pre = bq + xq @ wq.T
  out = rne(out_pre / s_o) * s_o   s_o = max(absmax(out_pre)/127, 1e-8)

Device strategy (column-parallel over out_features, 8 cores):
  - Quantized integers Qx in [-127,127] / Qw in [-7,7] are exact in bf16 and
    accumulate exactly in fp32 PSUM, so the matmul runs as an exact integer
    bf16 matmul; scales fold in afterwards: out_pre = (Qx@Qw)*(s_x*s_w) + bq.
  - Round-to-nearest-even via the fp32 magic constant (t + 1.5*2^23 then
    subtract), matching jnp.round exactly.
  - Pipeline per 512-token block: PE transposes raw fp32 x tiles into PSUM;
    ACT evicts with fused scale+magic (y = xT*inv_s + M); DVE finishes the
    round and converts to bf16 (qxT = y - M). No GPSIMD in the data path.
  - Global absmaxes via one tiny AllReduce-max up front (x-slice + w shard)
    and one for out_pre before the final requantization.
  - Each core computes out^T[j_shard, :] = [512, 4096]; host reassembles.
"""

import sys

sys.path.insert(0, "/opt/trn_rl_repo")

import numpy as np

import concourse.bass as bass
import concourse.mybir as mybir
import concourse.tile as tile
from concourse import bacc, bass_isa
from concourse.masks import make_identity

F32 = mybir.dt.float32
BF16 = mybir.dt.bfloat16
AF = mybir.ActivationFunctionType
ALU = mybir.AluOpType
AX = mybir.AxisListType

MAGIC = 12582912.0  # 1.5 * 2**23: fp32 add rounds to nearest-even integer
EPS = 1e-8
INV_QA = float(np.float32(1.0) / np.float32(127.0))
INV_QW = float(np.float32(1.0) / np.float32(7.0))

P = 128


def build(n_cores=8, T=4096, K=4096, J=4096, TB=512):
    JS = J // n_cores
    TS = T // n_cores
    n_kt = K // P
    n_tb = T // TB
    n_m = JS // P
    n_xs = TS // P
    XCH = min(2048, K)
    n_xch = K // XCH
    kpc = XCH // P  # k-tiles per x chunk

    nc = bacc.Bacc(
        "TRN2", target_bir_lowering=False, debug=False, num_devices=n_cores
    )

    x_d = nc.dram_tensor("x", [T, K], F32, kind="ExternalInput")
    w_d = nc.dram_tensor("w_shard", [JS, K], F32, kind="ExternalInput")
    b_d = nc.dram_tensor("b_full", [J], F32, kind="ExternalInput")
    bs_d = nc.dram_tensor("b_shard", [JS], F32, kind="ExternalInput")
    o_d = nc.dram_tensor("outT", [JS, T], F32, kind="ExternalOutput")
    cc1_in = nc.dram_tensor("cc1_in", [1, 2], F32)
    cc1_out = nc.dram_tensor("cc1_out", [1, 2], F32)
    cc2_in = nc.dram_tensor("cc2_in", [1, 1], F32)
    cc2_out = nc.dram_tensor("cc2_out", [1, 1], F32)
    groups = [list(range(n_cores))]

    with tile.TileContext(nc) as tc:
        with (
            tc.tile_pool(name="const", bufs=1) as const,
            tc.tile_pool(name="scal", bufs=1) as scal,
            tc.tile_pool(name="perm", bufs=1) as perm,
            tc.tile_pool(name="stage", bufs=6) as stage,
            tc.tile_pool(name="ypool", bufs=3) as ypool,
            tc.tile_pool(name="rpool", bufs=36) as rpool,
            tc.tile_pool(name="tps", bufs=3, space="PSUM") as tps,
            tc.tile_pool(name="mmps", bufs=5, space="PSUM") as mmps,
        ):
            identf = const.tile([P, P], F32)
            make_identity(nc, identf)
            magic_t = const.tile([P, 1], F32)
            nc.vector.memset(magic_t[:], MAGIC)

            # ---------------- Phase 0: absmax of the exclusive x slice -------
            # x is rotated per core on the host, so rows [0:TS) are this
            # core's exclusive absmax slice; w absmax rides on W-prep loads.
            nx = n_xs * n_xch
            am = scal.tile([P, nx + n_m * n_xch + 1], F32)
            for c in range(n_xs):
                for ch in range(n_xch):
                    t = stage.tile([P, XCH], F32, tag="xf")
                    nc.sync.dma_start(
                        t[:], x_d[c * P : (c + 1) * P, ch * XCH : (ch + 1) * XCH]
                    )
                    nc.vector.tensor_reduce(
                        am[:, c * n_xch + ch : c * n_xch + ch + 1], t[:],
                        axis=AX.X, op=ALU.max, apply_absolute_value=True,
                    )
            bfull = scal.tile([P, J // P], F32)
            nc.gpsimd.dma_start(bfull[:], b_d.rearrange("(p a) -> p a", p=P))
            nc.vector.tensor_reduce(
                am[:, nx + n_m * n_xch :], bfull[:], axis=AX.X, op=ALU.max,
                apply_absolute_value=True,
            )
            for c in range(n_m):
                for ch in range(n_xch):
                    t = stage.tile([P, XCH], F32, tag="xf")
                    nc.scalar.dma_start(
                        t[:], w_d[c * P : (c + 1) * P, ch * XCH : (ch + 1) * XCH]
                    )
                    nc.vector.tensor_reduce(
                        am[:, nx + c * n_xch + ch : nx + c * n_xch + ch + 1],
                        t[:], axis=AX.X, op=ALU.max, apply_absolute_value=True,
                    )

            m2 = scal.tile([P, 2], F32)
            nc.vector.tensor_reduce(m2[:, 0:1], am[:, :nx], axis=AX.X, op=ALU.max)
            nc.vector.tensor_reduce(
                m2[:, 1:2], am[:, nx : nx + n_m * n_xch], axis=AX.X, op=ALU.max
            )
            g2 = scal.tile([P, 2], F32)
            nc.gpsimd.partition_all_reduce(
                g2[:], m2[:], channels=P, reduce_op=bass_isa.ReduceOp.max
            )
            nc.sync.dma_start(cc1_in[:], g2[:1, :])
            nc.gpsimd.collective_compute(
                "AllReduce", ALU.max, replica_groups=groups,
                ins=[cc1_in[:]], outs=[cc1_out[:]],
            )
            gx = scal.tile([P, 2], F32)
            nc.sync.dma_start(gx[:1, :], cc1_out[:])
            bc2 = scal.tile([P, 2], F32)
            nc.gpsimd.partition_broadcast(bc2[:], gx[:1, :], channels=P)

            s_x = scal.tile([P, 1], F32)
            s_w = scal.tile([P, 1], F32)
            s_b = scal.tile([P, 1], F32)
            nc.vector.tensor_scalar(s_x[:], bc2[:, 0:1], INV_QA, EPS, op0=ALU.mult, op1=ALU.max)
            nc.vector.tensor_scalar(s_w[:], bc2[:, 1:2], INV_QW, EPS, op0=ALU.mult, op1=ALU.max)
            bmax = scal.tile([P, 1], F32)
            nc.gpsimd.partition_all_reduce(
                bmax[:], am[:, nx + n_m * n_xch :], channels=P, reduce_op=bass_isa.ReduceOp.max
            )
            nc.vector.tensor_scalar(s_b[:], bmax[:], INV_QA, EPS, op0=ALU.mult, op1=ALU.max)
            inv_sx = scal.tile([P, 1], F32)
            inv_sw = scal.tile([P, 1], F32)
            inv_sb = scal.tile([P, 1], F32)
            nc.vector.reciprocal(inv_sx[:], s_x[:])
            nc.vector.reciprocal(inv_sw[:], s_w[:])
            nc.vector.reciprocal(inv_sb[:], s_b[:])
            s_xw = scal.tile([P, 1], F32)
            nc.vector.tensor_tensor(out=s_xw[:], in0=s_x[:], in1=s_w[:], op=ALU.mult)

            bsh = scal.tile([P, n_m], F32)
            nc.gpsimd.dma_start(bsh[:], bs_d.rearrange("(a p) -> p a", p=P))
            by = scal.tile([P, n_m], F32)
            nc.scalar.activation(by[:], bsh[:], AF.Identity, bias=magic_t[:], scale=inv_sb[:])
            bq = scal.tile([P, n_m], F32)
            nc.vector.tensor_scalar(bq[:], by[:], -MAGIC, s_b[:], op0=ALU.add, op1=ALU.mult)

            # ---------------- W prep: transpose -> quantize -> QwT -----------
            qwT = perm.tile([P, n_kt, JS], BF16)  # [k%128, kt, j]
            for c in range(n_m):
                for ch in range(n_xch):
                    wf = stage.tile([P, XCH], F32, tag="xf")
                    nc.scalar.dma_start(
                        wf[:], w_d[c * P : (c + 1) * P, ch * XCH : (ch + 1) * XCH]
                    )
                    for kp in range(0, kpc, 2):
                        kt = ch * kpc + kp
                        pw = tps.tile([P, 2 * P], F32, tag="tp", name=f"pw_{c}_{kt}")
                        nc.tensor.transpose(
                            pw[:, 0:P], wf[:, kp * P : (kp + 1) * P], identf[:]
                        )
                        nc.tensor.transpose(
                            pw[:, P : 2 * P], wf[:, (kp + 1) * P : (kp + 2) * P], identf[:]
                        )
                        wy = ypool.tile([P, 2 * P], F32, tag="ysb")
                        nc.scalar.activation(
                            wy[:], pw[:], AF.Identity, bias=magic_t[:], scale=inv_sw[:]
                        )
                        wz = ypool.tile([P, 2 * P], BF16, tag="wz")
                        nc.vector.tensor_scalar(wz[:], wy[:], -MAGIC, None, op0=ALU.add)
                        nc.vector.tensor_copy(
                            out=qwT[:, kt, c * P : (c + 1) * P], in_=wz[:, 0:P]
                        )
                        nc.vector.tensor_copy(
                            out=qwT[:, kt + 1, c * P : (c + 1) * P], in_=wz[:, P : 2 * P]
                        )
            # ---------------- Main: transpose x -> quantize -> matmul --------
            opre = perm.tile([P, n_tb * n_m, TB], F32)
            omax = scal.tile([P, n_tb * n_m], F32)
            for tb in range(n_tb):
                qxT_t = []
                for kt in range(n_kt):
                    qxT_t.append(rpool.tile([P, TB], BF16, tag="qxT", name=f"qxT_{tb}_{kt}"))
                for half in range(TB // 256):
                    xf_t = {}
                    for tsh in range(2):
                        row0 = tb * TB + (half * 2 + tsh) * P
                        for ch in range(n_xch):
                            xf = stage.tile([P, XCH], F32, tag="xf")
                            nc.sync.dma_start(
                                xf[:], x_d[row0 : row0 + P, ch * XCH : (ch + 1) * XCH]
                            )
                            xf_t[(tsh, ch)] = xf
                    for kt in range(n_kt):
                        ch, kp = divmod(kt, kpc)
                        pt = tps.tile([P, 2 * P], F32, tag="tp")
                        nc.tensor.transpose(
                            pt[:, 0:P],
                            xf_t[(0, ch)][:, kp * P : (kp + 1) * P],
                            identf[:],
                        )
                        nc.tensor.transpose(
                            pt[:, P : 2 * P],
                            xf_t[(1, ch)][:, kp * P : (kp + 1) * P],
                            identf[:],
                        )
                        ysb = ypool.tile([P, 2 * P], F32, tag="ysb")
                        nc.scalar.activation(
                            ysb[:], pt[:], AF.Identity, bias=magic_t[:], scale=inv_sx[:]
                        )
                        nc.vector.tensor_scalar(
                            qxT_t[kt][:, half * 256 : half * 256 + 256],
                            ysb[:], -MAGIC, None, op0=ALU.add,
                        )
                ps_m = [mmps.tile([P, TB], F32, tag="mm", name=f"psmm_{tb}_{m}") for m in range(n_m)]
                for kt in range(n_kt):
                    for m in range(n_m):
                        nc.tensor.matmul(
                            ps_m[m][:],
                            lhsT=qwT[:, kt, m * P : (m + 1) * P],
                            rhs=qxT_t[kt][:],
                            start=(kt == 0),
                            stop=(kt == n_kt - 1),
                        )
                for m in range(n_m):
                    oc = opre[:, tb * n_m + m, :]
                    nc.scalar.activation(
                        oc, ps_m[m][:], AF.Identity, bias=bq[:, m : m + 1], scale=s_xw[:]
                    )
                    nc.vector.tensor_reduce(
                        omax[:, tb * n_m + m : tb * n_m + m + 1], oc,
                        axis=AX.X, op=ALU.max, apply_absolute_value=True,
                    )

            # ---------------- Tail: global out absmax -> requantize ---------
            om1 = scal.tile([P, 1], F32)
            nc.vector.tensor_reduce(om1[:], omax[:], axis=AX.X, op=ALU.max)
            omr = scal.tile([P, 1], F32)
            nc.gpsimd.partition_all_reduce(
                omr[:], om1[:], channels=P, reduce_op=bass_isa.ReduceOp.max
            )
            nc.sync.dma_start(cc2_in[:], omr[:1, :])
            nc.gpsimd.collective_compute(
                "AllReduce", ALU.max, replica_groups=groups,
                ins=[cc2_in[:]], outs=[cc2_out[:]],
            )
            go = scal.tile([P, 1], F32)
            nc.sync.dma_start(go[:1, :], cc2_out[:])
            bco = scal.tile([P, 1], F32)
            nc.gpsimd.partition_broadcast(bco[:], go[:1, :], channels=P)
            s_o = scal.tile([P, 1], F32)
            nc.vector.tensor_scalar(s_o[:], bco[:], INV_QA, EPS, op0=ALU.mult, op1=ALU.max)
            inv_so = scal.tile([P, 1], F32)
            nc.vector.reciprocal(inv_so[:], s_o[:])

            for tb in range(n_tb):
                for m in range(n_m):
                    oy = ypool.tile([P, TB], F32, tag="oy")
                    res = ypool.tile([P, TB], F32, tag="ores")
                    nc.scalar.activation(
                        oy[:], opre[:, tb * n_m + m, :], AF.Identity,
                        bias=magic_t[:], scale=inv_so[:],
                    )
                    nc.vector.tensor_scalar(res[:], oy[:], -MAGIC, s_o[:], op0=ALU.add, op1=ALU.mult)
                    nc.scalar.dma_start(
                        o_d[m * P : (m + 1) * P, tb * TB : (tb + 1) * TB], res[:]
                    )

    nc.compile()
    return nc


def _run(nc, inputs, n_cores, T, K, J, trace=False):
    from concourse.bass_utils import run_bass_kernel_spmd

    JS, TS = J // n_cores, T // n_cores
    x = np.ascontiguousarray(inputs["x"], dtype=np.float32)
    w = np.ascontiguousarray(inputs["weight"], dtype=np.float32)
    b = np.ascontiguousarray(inputs["b"], dtype=np.float32)
    in_maps = []
    for c in range(n_cores):
        in_maps.append(
            {
                # rotate so core c's exclusive absmax slice is its first block
                "x": np.roll(x, -c * TS, axis=0) if c else x,
                "w_shard": np.ascontiguousarray(w[c * JS : (c + 1) * JS]),
                "b_full": b,
                "b_shard": np.ascontiguousarray(b[c * JS : (c + 1) * JS]),
            }
        )
    res = run_bass_kernel_spmd(nc, in_maps, core_ids=list(range(n_cores)), trace=trace)
    shards = [np.roll(res.results[c]["outT"], c * TS, axis=1) for c in range(n_cores)]
    out = np.ascontiguousarray(np.concatenate(shards, axis=0).T)
    return out, res


_NC_CACHE = {}


def kernel(**inputs) -> np.ndarray:
    n_cores, T, K, J = 8, 4096, 4096, 4096
    key = (n_cores, T, K, J)
    if key not in _NC_CACHE:
        _NC_CACHE[key] = build(n_cores, T, K, J)
    out, _ = _run(_NC_CACHE[key], inputs, n_cores, T, K, J)
    return out
